# revision 3
# baseline (speedup 1.0000x reference)
"""GCNConv + GraphNorm + ReLU on 8 Trainium2 NeuronCores.

Strategy (graph-level data parallelism per the natural sharding):
  - Nodes are sharded by graph: core c owns graphs [16c, 16c+16) and their
    nodes (contiguous since batch_ptr is sorted). GraphNorm is then fully
    core-local.
  - Each core computes the full h = X @ W.T table (bf16) into its own HBM
    (replicated compute, ~21us of PE time), because its edges gather from
    arbitrary global src nodes.
  - Edges are bucketed by dst core on the host; per-edge normalization
    coefficients (symmetric GCN norm, incl. self loops) are computed on the
    host and folded into per-128-edge-block one-hot matrices.
  - Per core: bulk dma_gather of h[src] rows (edge-on-partition layout),
    then one matmul per 128-edge block scatters messages into a 128-column
    dst window of PSUM: out[h, c] += sum_e msg[e, h] * onehot[e, c], where
    onehot[e, col(dst_e)] = norm_e. Accumulated in fp32.
  - dma_gather indices are int16, so the h table is addressed in two halves
    (rows < 32768 and >= 32768); host groups each core's edges by half.
  - GraphNorm + ReLU run in the transposed [hidden, node-col] layout with
    per-graph fixed column slots; host un-shards/transposes at the end.
"""

import os
import sys

sys.path.insert(0, "/opt/trn_rl_repo")

import numpy as np
import ml_dtypes

import concourse.bass as bass
import concourse.bacc as bacc
import concourse.tile as tile
import concourse.mybir as mybir

BF16 = ml_dtypes.bfloat16

N_NODES = 50000
N_EDGES = 625000
HID = 128
NUM_GRAPHS = 128
NCORE = 8
GPC = NUM_GRAPHS // NCORE  # graphs per core
HALF = 32768  # int16 gather index limit
GCHUNK = 4096  # edge slots per dma_gather call (32 blocks)
EPS = 1e-5
P = 128


def _ceil(a, b):
    return -(-a // b)


class Meta:
    pass


def prep(node, edge_index, edge_attr, batch_ptr, W, b, gn_weight, gn_bias,
         gn_mean_scale, n_nodes=N_NODES, num_graphs=NUM_GRAPHS):
    """Host-side sharding/layout prep. Returns Meta with per-core arrays."""
    m = Meta()
    N = n_nodes
    NG = num_graphs
    src = np.asarray(edge_index[0]).astype(np.int64)
    dst = np.asarray(edge_index[1]).astype(np.int64)
    ew = np.asarray(edge_attr).astype(np.float32)
    batch = np.asarray(batch_ptr).astype(np.int64)
    node = np.asarray(node).astype(np.float32)
    W = np.asarray(W).astype(np.float32)
    b = np.asarray(b).astype(np.float32)
    gw = np.asarray(gn_weight).astype(np.float32)
    gb = np.asarray(gn_bias).astype(np.float32)
    gms = np.asarray(gn_mean_scale).astype(np.float32)

    # symmetric degree normalization with self loops (f32 like the reference)
    deg = (np.bincount(dst, weights=ew.astype(np.float64), minlength=N)
           .astype(np.float32) + np.float32(1.0))
    dinv = (1.0 / np.sqrt(deg)).astype(np.float32)
    esrc = np.concatenate([src, np.arange(N, dtype=np.int64)])
    edst = np.concatenate([dst, np.arange(N, dtype=np.int64)])
    enorm = np.concatenate([dinv[src] * ew * dinv[dst], dinv * dinv]).astype(np.float32)

    # graph layout: per-graph fixed column slots
    gstart = np.searchsorted(batch, np.arange(NG))
    gend = np.searchsorted(batch, np.arange(NG), side="right")
    cnt = (gend - gstart).astype(np.int64)
    GSLOT = int(_ceil(max(int(cnt.max()), 1), 8) * 8)
    NCOLR = GPC * GSLOT
    NWIN = _ceil(NCOLR, P)
    g_of = batch
    core_of_node = g_of // GPC
    col_of_node = (g_of % GPC) * GSLOT + (np.arange(N) - gstart[g_of])

    ecore = core_of_node[edst]
    ecol = col_of_node[edst]
    ewin = ecol // P
    ecolm = (ecol % P).astype(np.int64)
    ehalf = (esrc >= HALF).astype(np.int64)

    # per (core, win, half) counts -> shared per-window block capacities
    key = (ecore * NWIN + ewin) * 2 + ehalf
    counts = np.bincount(key, minlength=NCORE * NWIN * 2).reshape(NCORE, NWIN, 2)
    B0 = _ceil(counts[:, :, 0].max(axis=0), P).astype(np.int64)  # [NWIN] blocks
    B1 = _ceil(counts[:, :, 1].max(axis=0), P).astype(np.int64)
    blk_off0 = np.concatenate([[0], np.cumsum(B0)])
    blk_off1 = np.concatenate([[0], np.cumsum(B1)])
    NSLOT0 = int(blk_off0[-1]) * P
    NSLOT1 = int(blk_off1[-1]) * P
    NSLOT0P = _ceil(max(NSLOT0, 1), GCHUNK) * GCHUNK
    NSLOT1P = _ceil(max(NSLOT1, 1), GCHUNK) * GCHUNK
    NSLOTT = NSLOT0P + NSLOT1P
    NBLKT = NSLOTT // P

    # slot assignment: stable-sort edges by (core, half, win); position in group
    order = np.lexsort((ewin, ehalf, ecore))
    key2 = (ecore * 2 + ehalf) * NWIN + ewin
    k_sorted = key2[order]
    grp_change = np.concatenate([[True], k_sorted[1:] != k_sorted[:-1]])
    grp_first = np.where(grp_change)[0]
    grp_id = np.cumsum(grp_change) - 1
    pos_sorted = np.arange(len(order)) - grp_first[grp_id]
    pos = np.empty_like(pos_sorted)
    pos[order] = pos_sorted
    base0 = (blk_off0[:-1] * P)  # [NWIN]
    base1 = NSLOT0P + (blk_off1[:-1] * P)
    ebase = np.where(ehalf == 0, base0[ewin], base1[ewin])
    eslot = ebase + pos  # local slot within the owning core's slot array

    # per-core device arrays
    lidx = np.where(ehalf == 0, esrc, esrc - HALF).astype(np.int16)
    idx_arr = np.zeros((NCORE, NSLOTT), np.int16)
    idx_arr[ecore, eslot] = lidx
    oh = np.zeros((NCORE, NBLKT * P, P), dtype=BF16)
    oh[ecore, eslot, ecolm] = enorm
    oh = oh.reshape(NCORE, NBLKT, P, P)
    idx_wrap = idx_arr.reshape(NCORE, NSLOTT // 16, 16).transpose(0, 2, 1)
    idx_wrap = np.ascontiguousarray(np.tile(idx_wrap, (1, 8, 1)))  # [NCORE,128,S/16]

    # matmul inputs
    NPAD = _ceil(N, P) * P
    xt = np.zeros((HID, NPAD), dtype=BF16)
    xt[:, :N] = node.T.astype(BF16)
    wr = np.ascontiguousarray(W.T).astype(BF16)  # wr[k, j] = W[j, k]

    # GraphNorm constants
    cnt_core = cnt.reshape(NCORE, GPC).astype(np.float32)
    invc = (1.0 / np.maximum(cnt_core, 1.0)).astype(np.float32)  # [NCORE, GPC]
    invc_t = np.ascontiguousarray(
        np.broadcast_to(invc[:, None, :], (NCORE, P, GPC))).astype(np.float32)
    gnp = np.stack([gms, b * (1.0 - gms), gw, gb], axis=1).astype(np.float32)  # [128,4]

    # device block schedule: per global block: (half, win, first, last) or None
    sched = []
    for w in range(NWIN):
        for i in range(int(B0[w])):
            sched.append((0, w, i == 0, i == int(B0[w]) - 1))
    sched += [None] * ((NSLOT0P - NSLOT0) // P)
    for w in range(NWIN):
        for i in range(int(B1[w])):
            sched.append((1, w, i == 0, i == int(B1[w]) - 1))
    sched += [None] * ((NSLOT1P - NSLOT1) // P)
    assert len(sched) == NBLKT

    m.N, m.NG, m.NPAD = N, NG, NPAD
    m.GSLOT, m.NCOLR, m.NWIN = GSLOT, NCOLR, NWIN
    m.NSLOT0P, m.NSLOT1P, m.NSLOTT, m.NBLKT = NSLOT0P, NSLOT1P, NSLOTT, NBLKT
    m.sched = sched
    m.xt, m.wr, m.idx_wrap, m.oh, m.invc_t, m.gnp = xt, wr, idx_wrap, oh, invc_t, gnp
    m.gstart, m.cnt, m.col_of_node, m.core_of_node = gstart, cnt, col_of_node, core_of_node
    return m


def build_nc(m):
    """Build the per-core Bass program (SPMD: same NEFF on all 8 cores)."""
    nc = bacc.Bacc("TRN2", target_bir_lowering=False, debug=False)
    dt = mybir.dt

    xt_d = nc.dram_tensor("xt", [HID, m.NPAD], dt.bfloat16, kind="ExternalInput")
    wr_d = nc.dram_tensor("wr", [HID, HID], dt.bfloat16, kind="ExternalInput")
    idx_d = nc.dram_tensor("idx", [P, m.NSLOTT // 16], dt.int16, kind="ExternalInput")
    oh_d = nc.dram_tensor("oh", [m.NBLKT, P, P], dt.bfloat16, kind="ExternalInput")
    invc_d = nc.dram_tensor("invc", [P, GPC], dt.float32, kind="ExternalInput")
    gnp_d = nc.dram_tensor("gnp", [P, 4], dt.float32, kind="ExternalInput")
    out_d = nc.dram_tensor("outT", [P, m.NCOLR], dt.float32, kind="ExternalOutput")
    htab = nc.dram_tensor("htab", [m.NPAD, HID], dt.bfloat16)

    XBLK = m.NPAD // P
    HCH = 16  # node blocks per h-phase chunk
    hv = htab.ap().rearrange("(a p) d -> p a d", p=P)
    ohv = oh_d.ap().rearrange("n p w -> p n w")

    with tile.TileContext(nc) as tc:
        with tc.tile_pool(name="const", bufs=1) as cpool, \
             tc.tile_pool(name="xt", bufs=2) as xtp, \
             tc.tile_pool(name="hsb", bufs=2) as hsp, \
             tc.tile_pool(name="hps", bufs=2, space="PSUM") as hpsp, \
             tc.tile_pool(name="msg", bufs=2) as msgp, \
             tc.tile_pool(name="ohp", bufs=2) as ohp, \
             tc.tile_pool(name="wps", bufs=4, space="PSUM") as wpsp, \
             tc.tile_pool(name="acc", bufs=1) as accp, \
             tc.tile_pool(name="stat", bufs=1) as statp, \
             tc.tile_pool(name="sq", bufs=2) as sqp:

            wr_t = cpool.tile([HID, HID], dt.bfloat16)
            nc.sync.dma_start(wr_t[:], wr_d[:])
            idx_t = cpool.tile([P, m.NSLOTT // 16], dt.int16)
            nc.sync.dma_start(idx_t[:], idx_d[:])
            invc_t = cpool.tile([P, GPC], dt.float32)
            nc.sync.dma_start(invc_t[:], invc_d[:])
            gnp_t = cpool.tile([P, 4], dt.float32)
            nc.sync.dma_start(gnp_t[:], gnp_d[:])

            accT = accp.tile([P, m.NWIN * P], dt.float32)
            nc.vector.memset(accT[:], 0.0)

            # ---- Phase A: h table = (X @ W.T) rows, bf16, to HBM ----
            for c0 in range(0, XBLK, HCH):
                nb = min(HCH, XBLK - c0)
                xt_t = xtp.tile([P, HCH * P], dt.bfloat16, tag="xt")
                nc.sync.dma_start(xt_t[:, :nb * P], xt_d[:, c0 * P:(c0 + nb) * P])
                hs_t = hsp.tile([P, HCH, P], dt.bfloat16, tag="hs")
                for q0 in range(0, nb, 4):
                    qn = min(4, nb - q0)
                    hp_t = hpsp.tile([P, 4 * P], dt.float32, tag="hp", space="PSUM")
                    for i in range(qn):
                        nc.tensor.matmul(
                            hp_t[:, i * P:(i + 1) * P],
                            lhsT=xt_t[:, (q0 + i) * P:(q0 + i + 1) * P],
                            rhs=wr_t[:], start=True, stop=True)
                    nc.scalar.activation(
                        hs_t[:, q0:q0 + qn, :].rearrange("p a d -> p (a d)"),
                        hp_t[:, :qn * P],
                        mybir.ActivationFunctionType.Copy)
                nc.sync.dma_start(hv[:, c0:c0 + nb, :], hs_t[:, :nb, :])

            # ---- Phase B: gather + one-hot scatter matmuls ----
            tab0 = htab[0:HALF, :]
            tab1 = htab[HALF:m.NPAD, :]
            CB = GCHUNK // P  # blocks per gather chunk
            nchunk = m.NBLKT // CB
            psum_t = None
            for ch in range(nchunk):
                s_off = ch * GCHUNK
                in_half1 = s_off >= m.NSLOT0P
                tab = tab1 if in_half1 else tab0
                msg_t = msgp.tile([P, CB, P], dt.bfloat16, tag="msg")
                nc.gpsimd.dma_gather(
                    msg_t[:], tab[:],
                    idx_t[:, s_off // 16:(s_off + GCHUNK) // 16],
                    num_idxs=GCHUNK, num_idxs_reg=GCHUNK,
                    elem_size=HID, elem_step=HID, single_packet=False)
                oh_t = ohp.tile([P, CB, P], dt.bfloat16, tag="oh")
                nc.sync.dma_start(oh_t[:], ohv[:, ch * CB:(ch + 1) * CB, :])
                for j in range(CB):
                    ent = m.sched[ch * CB + j]
                    if ent is None:
                        continue
                    half, w, first, last = ent
                    if first:
                        psum_t = wpsp.tile([P, P], dt.float32, tag="wp", space="PSUM")
                    nc.tensor.matmul(
                        psum_t[:], lhsT=msg_t[:, j, :], rhs=oh_t[:, j, :],
                        start=first, stop=last)
                    if last:
                        nc.vector.tensor_tensor(
                            out=accT[:, w * P:(w + 1) * P],
                            in0=accT[:, w * P:(w + 1) * P],
                            in1=psum_t[:], op=mybir.AluOpType.add)

            # ---- Phase C: GraphNorm + ReLU (transposed layout) ----
            ms_ap = gnp_t[:, 0:1]
            cb_ap = gnp_t[:, 1:2]
            gw_ap = gnp_t[:, 2:3]
            gb_ap = gnp_t[:, 3:4]
            st = statp.tile([P, GPC * 8], dt.float32)
            sums = st[:, 0 * GPC:1 * GPC]
            q1 = st[:, 1 * GPC:2 * GPC]
            mu = st[:, 2 * GPC:3 * GPC]
            dd = st[:, 3 * GPC:4 * GPC]
            t1 = st[:, 4 * GPC:5 * GPC]
            var = st[:, 5 * GPC:6 * GPC]
            istd = st[:, 6 * GPC:7 * GPC]
            sh = st[:, 7 * GPC:8 * GPC]

            accR = accT[:, :m.NCOLR].rearrange("p (g s) -> p g s", g=GPC)
            nc.vector.tensor_reduce(sums, accR, axis=mybir.AxisListType.X,
                                    op=mybir.AluOpType.add)
            for g in range(GPC):
                sq_t = sqp.tile([P, m.GSLOT], dt.float32, tag="sq")
                nc.scalar.activation(
                    sq_t[:], accT[:, g * m.GSLOT:(g + 1) * m.GSLOT],
                    mybir.ActivationFunctionType.Square,
                    accum_out=q1[:, g:g + 1])
            # mu = sums * invc ; q = q1 * invc (q reuses q1)
            nc.vector.tensor_tensor(mu, sums, invc_t[:], op=mybir.AluOpType.mult)
            nc.vector.tensor_tensor(q1, q1, invc_t[:], op=mybir.AluOpType.mult)
            # d = ms*mu - cb
            nc.vector.tensor_scalar(dd, mu, ms_ap, cb_ap,
                                    op0=mybir.AluOpType.mult,
                                    op1=mybir.AluOpType.subtract)
            # t1 = 2*mu - d ; var = q - d*t1
            nc.vector.tensor_scalar(t1, mu, 2.0, None, op0=mybir.AluOpType.mult)
            nc.vector.tensor_tensor(t1, t1, dd, op=mybir.AluOpType.subtract)
            nc.vector.tensor_tensor(t1, t1, dd, op=mybir.AluOpType.mult)
            nc.vector.tensor_tensor(var, q1, t1, op=mybir.AluOpType.subtract)
            # istd = 1/sqrt(var+eps)
            eps_t = statp.tile([P, 1], dt.float32, tag="eps")
            nc.vector.memset(eps_t[:], float(EPS))
            nc.scalar.activation(istd, var, mybir.ActivationFunctionType.Sqrt,
                                 bias=eps_t[:])
            nc.vector.reciprocal(istd, istd)
            # scale = gw*istd (into istd); sh = gb - scale*d
            nc.vector.tensor_scalar(istd, istd, gw_ap, None,
                                    op0=mybir.AluOpType.mult)
            nc.vector.tensor_tensor(sh, istd, dd, op=mybir.AluOpType.mult)
            nc.vector.tensor_scalar(sh, sh, -1.0, gb_ap,
                                    op0=mybir.AluOpType.mult,
                                    op1=mybir.AluOpType.add)
            outT = accp.tile([P, m.NCOLR], dt.float32)
            for g in range(GPC):
                nc.scalar.activation(
                    outT[:, g * m.GSLOT:(g + 1) * m.GSLOT],
                    accT[:, g * m.GSLOT:(g + 1) * m.GSLOT],
                    mybir.ActivationFunctionType.Relu,
                    bias=sh[:, g:g + 1], scale=istd[:, g:g + 1])
            nc.sync.dma_start(out_d[:], outT[:])

    nc.compile()
    return nc


def in_maps_for(m):
    maps = []
    for c in range(NCORE):
        maps.append({
            "xt": m.xt,
            "wr": m.wr,
            "idx": m.idx_wrap[c],
            "oh": m.oh[c],
            "invc": m.invc_t[c],
            "gnp": m.gnp,
        })
    return maps


def unshard(m, outs):
    """outs: list of per-core {'outT': [128, NCOLR]} -> full [N, 128] f32."""
    res = np.empty((m.N, HID), dtype=np.float32)
    for c in range(NCORE):
        oT = outs[c]["outT"]
        for gl in range(GPC):
            g = c * GPC + gl
            n0 = int(m.gstart[g])
            k = int(m.cnt[g])
            if k:
                res[n0:n0 + k, :] = oT[:, gl * m.GSLOT:gl * m.GSLOT + k].T
    return res


def kernel(node, edge_index, edge_attr, batch_ptr, W, b, gn_weight, gn_bias,
           gn_mean_scale):
    from concourse import bass_utils
    m = prep(node, edge_index, edge_attr, batch_ptr, W, b, gn_weight, gn_bias,
             gn_mean_scale)
    nc = build_nc(m)
    res = bass_utils.run_bass_kernel_spmd(nc, in_maps_for(m),
                                          core_ids=list(range(NCORE)))
    return unshard(m, res.results)
